# revision 4
# baseline (speedup 1.0000x reference)
"""Trainium2 Bass kernel for nn_LogisticRegression (embedding_lookup), v2.

Reference computation (B=1024, S=200, V=50000, E=300):
    x1 = one-hot presence over vocab (duplicates set once)      [B, V]
    emb_mean = mean(emb_table[x], axis=1)                       [B, E]
    logits = concat([emb_mean, x1]) @ W.T + b                   [B, 1]
    out = sigmoid(logits)

Algebraic restructure:
    z[v]     = emb_table[v] . W_emb / S + W_voc[v]
    logit[r] = sum_v count[r,v] * z[v] - sum_{dup tokens} W_voc[v] + b
where count includes duplicates; the dup correction turns count*W_voc
into presence*W_voc.

Device plan (single NEFF, SPMD on 8 cores, vocab-sharded):
  core c owns vocab slice [6250c, 6250(c+1)), padded to 6272 = 128*49.
  phase 1: stream the 7.5MB table slice (contiguous 8.4KB/partition
           descriptors, v_loc = p*49+k) and fold W_emb/S with fused
           scalar_tensor_tensor accumulations -> t[128, 49]; add the
           W_voc slice -> z[128, 49]; convert to bf16 per chunk.
  phase 2: logits_partial[1, 1024] = sum_k z[:, k]^T @ P[:, k, :]
           on the TensorEngine, where P[p, k, r] (fp8, exact small
           counts) is the host-built count matrix for this vocab
           slice (6.4MB, streamed concurrently with the table).
  dup fix: one extra contraction step: stationary = -W_voc values at
           this core's rows' dup tokens (host-indexed input, <=128
           slots), moving = host one-hot Q[slot, row] fp8. Rides the
           same PSUM accumulation, so no gather and no tail work.
  ReduceScatter(add) over the 8 partial logit vectors -> each core
           holds final logits for its own 128 batch rows.
  tail:    sigmoid(logit + b) -> outp[128, 1]; host concatenates.

Why not index-gathers: dma_gather generates descriptors at ~8.4ns/idx
on 2 of 8 GpSimd Q7 cores (215us for 25600 tokens) and ap_gather is
read-latency-bound at ~27ns/idx; the dense fp8 count-matrix streams at
full DMA bandwidth instead and the matmul is essentially free.
"""

import sys

if "/opt/trn_rl_repo" not in sys.path:
    sys.path.insert(0, "/opt/trn_rl_repo")

# This image's antenv package lacks the optional axon_hooks module, but
# concourse.bass_utils imports it unconditionally on the BASS_TRACE path.
try:
    import antenv.axon_hooks  # noqa: F401
except ImportError:
    import types as _types

    import antenv as _antenv

    _hooks_mod = _types.ModuleType("antenv.axon_hooks")
    _hooks_mod._hook = None

    def _set_hook(h, _m=_hooks_mod):
        _m._hook = h

    def _get_hook(_m=_hooks_mod):
        return _m._hook

    _hooks_mod.set_axon_ntff_profile_hook = _set_hook
    _hooks_mod.get_axon_ntff_profile_hook = _get_hook
    sys.modules["antenv.axon_hooks"] = _hooks_mod
    _antenv.axon_hooks = _hooks_mod

import ml_dtypes
import numpy as np

from concourse import bacc, mybir, tile
from concourse.bass_utils import run_bass_kernel_spmd

N_CORES = 8
B = 1024
S = 200
V = 50000
E = 300

RPC = B // N_CORES          # batch rows per core = 128
VPC = V // N_CORES          # true vocab rows per core = 6250
KC = 49                     # t columns: v_loc = p*49 + k
VPAD = KC * 128             # padded vocab rows per core = 6272
TCHUNK = 7                  # table rows-per-partition per DMA chunk
NCHUNK = KC // TCHUNK       # 7 chunks
HALF = 512                  # moving free dim per matmul (PSUM bank limit)

_BUILT = None
LAST_RUN = None  # BassKernelResults of the most recent launch (for harness)


def _build():
    f32 = mybir.dt.float32
    bf16 = mybir.dt.bfloat16
    fp8 = mybir.dt.float8e4
    nc = bacc.Bacc("TRN2", target_bir_lowering=False, debug=False,
                   num_devices=N_CORES)

    tbl = nc.dram_tensor("tbl", [VPAD, E], f32, kind="ExternalInput")
    wemb = nc.dram_tensor("wemb", [1, E], f32, kind="ExternalInput")
    wvoc = nc.dram_tensor("wvoc", [128, KC], f32, kind="ExternalInput")
    pmat = nc.dram_tensor("pmat", [128, KC, B], fp8, kind="ExternalInput")
    wdup = nc.dram_tensor("wdup", [128, 1], f32, kind="ExternalInput")
    qmat = nc.dram_tensor("qmat", [128, B], fp8, kind="ExternalInput")
    bias = nc.dram_tensor("bias", [1, 1], f32, kind="ExternalInput")
    outp = nc.dram_tensor("outp", [RPC, N_CORES], f32, kind="ExternalOutput")

    with tile.TileContext(nc) as tc:
        with tc.tile_pool(name="dram", bufs=1, space="DRAM") as dram, \
             tc.tile_pool(name="sbuf", bufs=1) as sb1, \
             tc.tile_pool(name="ld", bufs=4) as ld, \
             tc.tile_pool(name="scr", bufs=2) as scr, \
             tc.psum_pool(name="psum", bufs=1) as pp:
            partial_d = dram.tile([B], f32)
            allred_d = dram.tile([B], f32, addr_space="Shared")

            # --- small loads (wemb/wvoc first: they gate the STT chain) ---
            wemb_sb = sb1.tile([128, E], f32)
            nc.sync.dma_start(wemb_sb[:], wemb.ap().partition_broadcast(128))
            nc.vector.tensor_scalar_mul(wemb_sb[:], wemb_sb[:], 1.0 / S)
            wemb_bf = sb1.tile([128, E], bf16)
            nc.vector.tensor_copy(out=wemb_bf[:], in_=wemb_sb[:])
            wvoc_sb = sb1.tile([128, KC], f32)
            nc.sync.dma_start(wvoc_sb[:], wvoc.ap())
            wdup_sb = sb1.tile([128, 1], f32)
            nc.scalar.dma_start(wdup_sb[:], wdup.ap())
            qm = sb1.tile([128, B], fp8)
            nc.scalar.dma_start(qm[:], qmat.ap())
            b_sb = sb1.tile([128, 1], f32)
            nc.scalar.dma_start(b_sb[:], bias.ap().partition_broadcast(128))
            wneg = sb1.tile([128, 1, 1], bf16)
            nc.vector.tensor_scalar_mul(wneg[:, 0, :], wdup_sb[:], -1.0)

            # warm the sigmoid activation table before the critical tail
            warm = scr.tile([128, 1], f32, tag="warm")
            nc.scalar.activation(
                out=warm[:], in_=b_sb[:],
                func=mybir.ActivationFunctionType.Sigmoid, scale=1.0)

            # --- P matrix: stream in k-chunks alongside the table ---
            pm = sb1.tile([128, KC, B], fp8)
            for ch in range(NCHUNK):
                nc.scalar.dma_start(
                    pm[:, TCHUNK * ch:TCHUNK * (ch + 1), :],
                    pmat.ap()[:, TCHUNK * ch:TCHUNK * (ch + 1), :])

            # --- phase 1 + fused phase 2 matmuls per chunk ---
            u = sb1.tile([128, KC], f32)          # t values
            zb = sb1.tile([128, KC, 1], bf16)     # z = t + wvoc, bf16
            psum_a = pp.tile([1, HALF], f32)
            psum_b = pp.tile([1, HALF], f32)
            tview = tbl.ap().bitcast(bf16).rearrange(
                "(p k) (e two) -> p k e two", p=128, two=2)
            for ch in range(NCHUNK):
                k0 = TCHUNK * ch
                chunk = ld.tile([128, TCHUNK, E], bf16, tag="tblchunk")
                nc.sync.dma_start(chunk[:], tview[:, k0:k0 + TCHUNK, :, 1])
                for c in range(TCHUNK):
                    po = scr.tile([128, E], bf16, tag="po")
                    nc.vector.scalar_tensor_tensor(
                        out=po[:], in0=chunk[:, c, :], scalar=1.0,
                        in1=wemb_bf[:],
                        op0=mybir.AluOpType.mult, op1=mybir.AluOpType.mult,
                        accum_out=u[:, k0 + c:k0 + c + 1])
                zch = scr.tile([128, TCHUNK], f32, tag="zch")
                nc.vector.tensor_tensor(
                    out=zch[:], in0=u[:, k0:k0 + TCHUNK],
                    in1=wvoc_sb[:, k0:k0 + TCHUNK], op=mybir.AluOpType.add)
                nc.vector.tensor_copy(out=zb[:, k0:k0 + TCHUNK, 0], in_=zch[:])
                for c in range(TCHUNK):
                    k = k0 + c
                    nc.tensor.matmul(
                        psum_a[:], zb[:, k, :], pm[:, k, :HALF],
                        start=(k == 0), stop=False)
                    nc.tensor.matmul(
                        psum_b[:], zb[:, k, :], pm[:, k, HALF:],
                        start=(k == 0), stop=False)

            # dup correction: one more contraction step on the same PSUM
            nc.tensor.matmul(
                psum_a[:], wneg[:, 0, :], qm[:, :HALF],
                start=False, stop=True)
            nc.tensor.matmul(
                psum_b[:], wneg[:, 0, :], qm[:, HALF:],
                start=False, stop=True)

            partial_sb = sb1.tile([1, B], f32)
            nc.vector.tensor_copy(out=partial_sb[:, :HALF], in_=psum_a[:])
            nc.vector.tensor_copy(out=partial_sb[:, HALF:], in_=psum_b[:])
            nc.scalar.dma_start(partial_d[:], partial_sb[:])

            nc.gpsimd.collective_compute(
                "AllReduce",
                mybir.AluOpType.add,
                replica_groups=[list(range(N_CORES))],
                ins=[partial_d.opt()],
                outs=[allred_d.opt()],
            )

            mine = sb1.tile([RPC, N_CORES], f32)
            nc.scalar.dma_start(
                mine[:], allred_d[:].rearrange("(hi p) -> p hi", p=128))
            res = sb1.tile([RPC, N_CORES], f32)
            nc.scalar.activation(
                out=res[:], in_=mine[:],
                func=mybir.ActivationFunctionType.Sigmoid,
                bias=b_sb[:], scale=1.0)
            nc.scalar.dma_start(outp.ap(), res[:])

    nc.compile()
    return nc


def _first_occurrence_mask(xr: np.ndarray) -> np.ndarray:
    """m[i, j] = 1 iff x[i, j] does not appear at any k < j in row i."""
    eq = xr[:, :, None] == xr[:, None, :]          # [rows, S, S]
    dup = np.tril(eq, -1).any(axis=2)              # seen earlier in the row
    return ~dup


def kernel(x, emb_table, W, b):
    global _BUILT, LAST_RUN
    if _BUILT is None:
        _BUILT = _build()
    nc = _BUILT

    x = np.asarray(x)
    emb_table = np.ascontiguousarray(np.asarray(emb_table, dtype=np.float32))
    W = np.asarray(W, dtype=np.float32)
    b = np.asarray(b, dtype=np.float32)

    wemb = np.ascontiguousarray(W[:, :E])                  # [1, E]
    wv_full = W[0, E:]                                     # [V]
    bias_np = b.reshape(1, 1)

    xl = x.astype(np.int64)                                # [B, S]
    dup = ~_first_occurrence_mask(xl)                      # [B, S] bool

    in_maps = []
    for c in range(N_CORES):
        tbl = np.zeros((VPAD, E), dtype=np.float32)
        tbl[:VPC] = emb_table[c * VPC:(c + 1) * VPC]
        wvs = np.zeros(VPAD, dtype=np.float32)
        wvs[:VPC] = wv_full[c * VPC:(c + 1) * VPC]
        wvoc_sh = wvs.reshape(128, KC)                     # v_loc = p*49 + k

        # count matrix for this vocab slice over ALL batch rows
        in_slice = (xl >= c * VPC) & (xl < (c + 1) * VPC)
        rr, jj = np.nonzero(in_slice)
        vloc = xl[rr, jj] - c * VPC
        pcnt = np.zeros((VPAD, B), dtype=np.int32)
        np.add.at(pcnt, (vloc, rr), 1)
        assert pcnt.max() <= 15, "counts exceed exact fp8e4 range"
        pmat_np = pcnt.astype(ml_dtypes.float8_e4m3).reshape(128, KC, B)

        # dup-correction: -W_voc at dup tokens of this core's OWN rows,
        # one PSUM contraction step (host only INDEXES W, no arithmetic)
        rows = slice(c * RPC, (c + 1) * RPC)
        dr, dj = np.nonzero(dup[rows])                     # local row, seq pos
        dv = xl[rows][dr, dj]                              # global vocab id
        assert len(dr) <= 128, "more than 128 dup tokens on one core"
        wdup_np = np.zeros((128, 1), dtype=np.float32)
        qcnt = np.zeros((128, B), dtype=np.int32)
        for s, (r, v) in enumerate(zip(dr, dv)):
            wdup_np[s, 0] = wv_full[v]
            qcnt[s, c * RPC + r] += 1
        assert qcnt.max() <= 15
        qmat_np = qcnt.astype(ml_dtypes.float8_e4m3)

        in_maps.append({
            "tbl": tbl,
            "wemb": wemb,
            "wvoc": wvoc_sh,
            "pmat": pmat_np,
            "wdup": wdup_np,
            "qmat": qmat_np,
            "bias": bias_np,
        })

    LAST_RUN = run_bass_kernel_spmd(nc, in_maps, core_ids=list(range(N_CORES)))
    out = np.concatenate(
        [LAST_RUN.results[c]["outp"][:, c] for c in range(N_CORES)]
    )
    return out.reshape(B, 1)


# revision 5
# speedup vs baseline: 18.2046x; 18.2046x over previous
"""Trainium2 Bass kernel for nn_LogisticRegression (embedding_lookup), v2.

Reference computation (B=1024, S=200, V=50000, E=300):
    x1 = one-hot presence over vocab (duplicates set once)      [B, V]
    emb_mean = mean(emb_table[x], axis=1)                       [B, E]
    logits = concat([emb_mean, x1]) @ W.T + b                   [B, 1]
    out = sigmoid(logits)

Algebraic restructure:
    z[v]     = emb_table[v] . W_emb / S + W_voc[v]
    logit[r] = sum_v count[r,v] * z[v] - sum_{dup tokens} W_voc[v] + b
where count includes duplicates; the dup correction turns count*W_voc
into presence*W_voc.

Device plan (single NEFF, SPMD on 8 cores, vocab-sharded):
  core c owns vocab slice [6250c, 6250(c+1)), padded to 6272 = 128*49.
  phase 1: stream the 7.5MB table slice (contiguous 8.4KB/partition
           descriptors, v_loc = p*49+k) and fold W_emb/S with fused
           scalar_tensor_tensor accumulations -> t[128, 49]; add the
           W_voc slice -> z[128, 49]; convert to bf16 per chunk.
  phase 2: logits_partial[1, 1024] = sum_k z[:, k]^T @ P[:, k, :]
           on the TensorEngine, where P[p, k, r] (fp8, exact small
           counts) is the host-built count matrix for this vocab
           slice (6.4MB, streamed concurrently with the table).
  dup fix: one extra contraction step: stationary = -W_voc values at
           this core's rows' dup tokens (host-indexed input, <=128
           slots), moving = host one-hot Q[slot, row] fp8. Rides the
           same PSUM accumulation, so no gather and no tail work.
  ReduceScatter(add) over the 8 partial logit vectors -> each core
           holds final logits for its own 128 batch rows.
  tail:    sigmoid(logit + b) -> outp[128, 1]; host concatenates.

Why not index-gathers: dma_gather generates descriptors at ~8.4ns/idx
on 2 of 8 GpSimd Q7 cores (215us for 25600 tokens) and ap_gather is
read-latency-bound at ~27ns/idx; the dense fp8 count-matrix streams at
full DMA bandwidth instead and the matmul is essentially free.
"""

import sys

if "/opt/trn_rl_repo" not in sys.path:
    sys.path.insert(0, "/opt/trn_rl_repo")

# This image's antenv package lacks the optional axon_hooks module, but
# concourse.bass_utils imports it unconditionally on the BASS_TRACE path.
try:
    import antenv.axon_hooks  # noqa: F401
except ImportError:
    import types as _types

    import antenv as _antenv

    _hooks_mod = _types.ModuleType("antenv.axon_hooks")
    _hooks_mod._hook = None

    def _set_hook(h, _m=_hooks_mod):
        _m._hook = h

    def _get_hook(_m=_hooks_mod):
        return _m._hook

    _hooks_mod.set_axon_ntff_profile_hook = _set_hook
    _hooks_mod.get_axon_ntff_profile_hook = _get_hook
    sys.modules["antenv.axon_hooks"] = _hooks_mod
    _antenv.axon_hooks = _hooks_mod

import ml_dtypes
import numpy as np

from concourse import bacc, mybir, tile
from concourse.bass_utils import run_bass_kernel_spmd

N_CORES = 8
B = 1024
S = 200
V = 50000
E = 300

RPC = B // N_CORES          # batch rows per core = 128
VPC = V // N_CORES          # true vocab rows per core = 6250
KC = 49                     # t columns: v_loc = p*49 + k
VPAD = KC * 128             # padded vocab rows per core = 6272
TCHUNK = 7                  # table rows-per-partition per DMA chunk
NCHUNK = KC // TCHUNK       # 7 chunks
HALF = 512                  # moving free dim per matmul (PSUM bank limit)

_BUILT = None
LAST_RUN = None  # BassKernelResults of the most recent launch (for harness)


def _build():
    f32 = mybir.dt.float32
    bf16 = mybir.dt.bfloat16
    fp8 = mybir.dt.float8e4
    nc = bacc.Bacc("TRN2", target_bir_lowering=False, debug=False,
                   num_devices=N_CORES)

    tbl = nc.dram_tensor("tbl", [VPAD, E], bf16, kind="ExternalInput")
    wemb = nc.dram_tensor("wemb", [1, E], f32, kind="ExternalInput")
    wvoc = nc.dram_tensor("wvoc", [128, KC], f32, kind="ExternalInput")
    pmat = nc.dram_tensor("pmat", [128, KC, B], fp8, kind="ExternalInput")
    wdup = nc.dram_tensor("wdup", [128, 1], f32, kind="ExternalInput")
    qmat = nc.dram_tensor("qmat", [128, B], fp8, kind="ExternalInput")
    bias = nc.dram_tensor("bias", [1, 1], f32, kind="ExternalInput")
    outp = nc.dram_tensor("outp", [RPC, N_CORES], f32, kind="ExternalOutput")

    with tile.TileContext(nc) as tc:
        with tc.tile_pool(name="dram", bufs=1, space="DRAM") as dram, \
             tc.tile_pool(name="sbuf", bufs=1) as sb1, \
             tc.tile_pool(name="ld", bufs=4) as ld, \
             tc.tile_pool(name="scr", bufs=2) as scr, \
             tc.psum_pool(name="psum", bufs=1) as pp:
            partial_d = dram.tile([B], f32)
            allred_d = dram.tile([B], f32, addr_space="Shared")

            # --- small loads (wemb/wvoc first: they gate the STT chain) ---
            wemb_sb = sb1.tile([128, E], f32)
            nc.sync.dma_start(wemb_sb[:], wemb.ap().partition_broadcast(128))
            nc.vector.tensor_scalar_mul(wemb_sb[:], wemb_sb[:], 1.0 / S)
            wemb_bf = sb1.tile([128, E], bf16)
            nc.vector.tensor_copy(out=wemb_bf[:], in_=wemb_sb[:])
            wvoc_sb = sb1.tile([128, KC], f32)
            nc.sync.dma_start(wvoc_sb[:], wvoc.ap())
            wdup_sb = sb1.tile([128, 1], f32)
            nc.scalar.dma_start(wdup_sb[:], wdup.ap())
            qm = sb1.tile([128, B], fp8)
            nc.scalar.dma_start(qm[:], qmat.ap())
            b_sb = sb1.tile([128, 1], f32)
            nc.scalar.dma_start(b_sb[:], bias.ap().partition_broadcast(128))
            wneg = sb1.tile([128, 1, 1], bf16)
            nc.vector.tensor_scalar_mul(wneg[:, 0, :], wdup_sb[:], -1.0)

            # warm the sigmoid activation table before the critical tail
            warm = scr.tile([128, 1], f32, tag="warm")
            nc.scalar.activation(
                out=warm[:], in_=b_sb[:],
                func=mybir.ActivationFunctionType.Sigmoid, scale=1.0)

            # --- P matrix: stream in k-chunks alongside the table ---
            pm = sb1.tile([128, KC, B], fp8)
            for ch in range(NCHUNK):
                nc.scalar.dma_start(
                    pm[:, TCHUNK * ch:TCHUNK * (ch + 1), :],
                    pmat.ap()[:, TCHUNK * ch:TCHUNK * (ch + 1), :])

            # --- phase 1 + fused phase 2 matmuls per chunk ---
            u = sb1.tile([128, KC], f32)          # t values
            zb = sb1.tile([128, KC, 1], bf16)     # z = t + wvoc, bf16
            psum_a = pp.tile([1, HALF], f32)
            psum_b = pp.tile([1, HALF], f32)
            tview = tbl.ap().rearrange("(p k) e -> p k e", p=128)
            for ch in range(NCHUNK):
                k0 = TCHUNK * ch
                chunk = ld.tile([128, TCHUNK, E], bf16, tag="tblchunk")
                nc.sync.dma_start(chunk[:], tview[:, k0:k0 + TCHUNK, :])
                for c in range(TCHUNK):
                    po = scr.tile([128, E], bf16, tag="po")
                    nc.vector.scalar_tensor_tensor(
                        out=po[:], in0=chunk[:, c, :], scalar=1.0,
                        in1=wemb_bf[:],
                        op0=mybir.AluOpType.mult, op1=mybir.AluOpType.mult,
                        accum_out=u[:, k0 + c:k0 + c + 1])
                zch = scr.tile([128, TCHUNK], f32, tag="zch")
                nc.vector.tensor_tensor(
                    out=zch[:], in0=u[:, k0:k0 + TCHUNK],
                    in1=wvoc_sb[:, k0:k0 + TCHUNK], op=mybir.AluOpType.add)
                nc.vector.tensor_copy(out=zb[:, k0:k0 + TCHUNK, 0], in_=zch[:])
                for c in range(TCHUNK):
                    k = k0 + c
                    nc.tensor.matmul(
                        psum_a[:], zb[:, k, :], pm[:, k, :HALF],
                        start=(k == 0), stop=False)
                    nc.tensor.matmul(
                        psum_b[:], zb[:, k, :], pm[:, k, HALF:],
                        start=(k == 0), stop=False)

            # dup correction: one more contraction step on the same PSUM
            nc.tensor.matmul(
                psum_a[:], wneg[:, 0, :], qm[:, :HALF],
                start=False, stop=True)
            nc.tensor.matmul(
                psum_b[:], wneg[:, 0, :], qm[:, HALF:],
                start=False, stop=True)

            partial_sb = sb1.tile([1, B], f32)
            nc.vector.tensor_copy(out=partial_sb[:, :HALF], in_=psum_a[:])
            nc.vector.tensor_copy(out=partial_sb[:, HALF:], in_=psum_b[:])
            nc.scalar.dma_start(partial_d[:], partial_sb[:])

            nc.gpsimd.collective_compute(
                "AllReduce",
                mybir.AluOpType.add,
                replica_groups=[list(range(N_CORES))],
                ins=[partial_d.opt()],
                outs=[allred_d.opt()],
            )

            mine = sb1.tile([RPC, N_CORES], f32)
            nc.scalar.dma_start(
                mine[:], allred_d[:].rearrange("(hi p) -> p hi", p=128))
            res = sb1.tile([RPC, N_CORES], f32)
            nc.scalar.activation(
                out=res[:], in_=mine[:],
                func=mybir.ActivationFunctionType.Sigmoid,
                bias=b_sb[:], scale=1.0)
            nc.scalar.dma_start(outp.ap(), res[:])

    nc.compile()
    return nc


def _first_occurrence_mask(xr: np.ndarray) -> np.ndarray:
    """m[i, j] = 1 iff x[i, j] does not appear at any k < j in row i."""
    eq = xr[:, :, None] == xr[:, None, :]          # [rows, S, S]
    dup = np.tril(eq, -1).any(axis=2)              # seen earlier in the row
    return ~dup


def kernel(x, emb_table, W, b):
    global _BUILT, LAST_RUN
    if _BUILT is None:
        _BUILT = _build()
    nc = _BUILT

    x = np.asarray(x)
    emb_table = np.ascontiguousarray(np.asarray(emb_table, dtype=np.float32))
    W = np.asarray(W, dtype=np.float32)
    b = np.asarray(b, dtype=np.float32)

    wemb = np.ascontiguousarray(W[:, :E])                  # [1, E]
    wv_full = W[0, E:]                                     # [V]
    bias_np = b.reshape(1, 1)

    xl = x.astype(np.int64)                                # [B, S]
    dup = ~_first_occurrence_mask(xl)                      # [B, S] bool

    in_maps = []
    for c in range(N_CORES):
        tbl = np.zeros((VPAD, E), dtype=ml_dtypes.bfloat16)
        tbl[:VPC] = emb_table[c * VPC:(c + 1) * VPC]
        wvs = np.zeros(VPAD, dtype=np.float32)
        wvs[:VPC] = wv_full[c * VPC:(c + 1) * VPC]
        wvoc_sh = wvs.reshape(128, KC)                     # v_loc = p*49 + k

        # count matrix for this vocab slice over ALL batch rows
        in_slice = (xl >= c * VPC) & (xl < (c + 1) * VPC)
        rr, jj = np.nonzero(in_slice)
        vloc = xl[rr, jj] - c * VPC
        pcnt = np.zeros((VPAD, B), dtype=np.int32)
        np.add.at(pcnt, (vloc, rr), 1)
        assert pcnt.max() <= 15, "counts exceed exact fp8e4 range"
        pmat_np = pcnt.astype(ml_dtypes.float8_e4m3).reshape(128, KC, B)

        # dup-correction: -W_voc at dup tokens of this core's OWN rows,
        # one PSUM contraction step (host only INDEXES W, no arithmetic)
        rows = slice(c * RPC, (c + 1) * RPC)
        dr, dj = np.nonzero(dup[rows])                     # local row, seq pos
        dv = xl[rows][dr, dj]                              # global vocab id
        assert len(dr) <= 128, "more than 128 dup tokens on one core"
        wdup_np = np.zeros((128, 1), dtype=np.float32)
        qcnt = np.zeros((128, B), dtype=np.int32)
        for s, (r, v) in enumerate(zip(dr, dv)):
            wdup_np[s, 0] = wv_full[v]
            qcnt[s, c * RPC + r] += 1
        assert qcnt.max() <= 15
        qmat_np = qcnt.astype(ml_dtypes.float8_e4m3)

        in_maps.append({
            "tbl": tbl,
            "wemb": wemb,
            "wvoc": wvoc_sh,
            "pmat": pmat_np,
            "wdup": wdup_np,
            "qmat": qmat_np,
            "bias": bias_np,
        })

    LAST_RUN = run_bass_kernel_spmd(nc, in_maps, core_ids=list(range(N_CORES)))
    out = np.concatenate(
        [LAST_RUN.results[c]["outp"][:, c] for c in range(N_CORES)]
    )
    return out.reshape(B, 1)


# revision 6
# speedup vs baseline: 18.7038x; 1.0274x over previous
"""Trainium2 Bass kernel for nn_LogisticRegression (embedding_lookup), v2.

Reference computation (B=1024, S=200, V=50000, E=300):
    x1 = one-hot presence over vocab (duplicates set once)      [B, V]
    emb_mean = mean(emb_table[x], axis=1)                       [B, E]
    logits = concat([emb_mean, x1]) @ W.T + b                   [B, 1]
    out = sigmoid(logits)

Algebraic restructure:
    z[v]     = emb_table[v] . W_emb / S + W_voc[v]
    logit[r] = sum_v count[r,v] * z[v] - sum_{dup tokens} W_voc[v] + b
where count includes duplicates; the dup correction turns count*W_voc
into presence*W_voc.

Device plan (single NEFF, SPMD on 8 cores, vocab-sharded):
  core c owns vocab slice [6250c, 6250(c+1)), padded to 6272 = 128*49.
  phase 1: stream the 7.5MB table slice (contiguous 8.4KB/partition
           descriptors, v_loc = p*49+k) and fold W_emb/S with fused
           scalar_tensor_tensor accumulations -> t[128, 49]; add the
           W_voc slice -> z[128, 49]; convert to bf16 per chunk.
  phase 2: logits_partial[1, 1024] = sum_k z[:, k]^T @ P[:, k, :]
           on the TensorEngine, where P[p, k, r] (fp8, exact small
           counts) is the host-built count matrix for this vocab
           slice (6.4MB, streamed concurrently with the table).
  dup fix: one extra contraction step: stationary = -W_voc values at
           this core's rows' dup tokens (host-indexed input, <=128
           slots), moving = host one-hot Q[slot, row] fp8. Rides the
           same PSUM accumulation, so no gather and no tail work.
  ReduceScatter(add) over the 8 partial logit vectors -> each core
           holds final logits for its own 128 batch rows.
  tail:    sigmoid(logit + b) -> outp[128, 1]; host concatenates.

Why not index-gathers: dma_gather generates descriptors at ~8.4ns/idx
on 2 of 8 GpSimd Q7 cores (215us for 25600 tokens) and ap_gather is
read-latency-bound at ~27ns/idx; the dense fp8 count-matrix streams at
full DMA bandwidth instead and the matmul is essentially free.
"""

import sys

if "/opt/trn_rl_repo" not in sys.path:
    sys.path.insert(0, "/opt/trn_rl_repo")

# This image's antenv package lacks the optional axon_hooks module, but
# concourse.bass_utils imports it unconditionally on the BASS_TRACE path.
try:
    import antenv.axon_hooks  # noqa: F401
except ImportError:
    import types as _types

    import antenv as _antenv

    _hooks_mod = _types.ModuleType("antenv.axon_hooks")
    _hooks_mod._hook = None

    def _set_hook(h, _m=_hooks_mod):
        _m._hook = h

    def _get_hook(_m=_hooks_mod):
        return _m._hook

    _hooks_mod.set_axon_ntff_profile_hook = _set_hook
    _hooks_mod.get_axon_ntff_profile_hook = _get_hook
    sys.modules["antenv.axon_hooks"] = _hooks_mod
    _antenv.axon_hooks = _hooks_mod

import ml_dtypes
import numpy as np

from concourse import bacc, mybir, tile
from concourse.bass_utils import run_bass_kernel_spmd

N_CORES = 8
B = 1024
S = 200
V = 50000
E = 300

RPC = B // N_CORES          # batch rows per core = 128
VPC = V // N_CORES          # true vocab rows per core = 6250
KC = 49                     # t columns: v_loc = p*49 + k
VPAD = KC * 128             # padded vocab rows per core = 6272
TCHUNK = 7                  # table rows-per-partition per DMA chunk
NCHUNK = KC // TCHUNK       # 7 chunks
HALF = 512                  # moving free dim per matmul (PSUM bank limit)

_BUILT = None
LAST_RUN = None  # BassKernelResults of the most recent launch (for harness)


def _build():
    f32 = mybir.dt.float32
    bf16 = mybir.dt.bfloat16
    fp8 = mybir.dt.float8e4
    nc = bacc.Bacc("TRN2", target_bir_lowering=False, debug=False,
                   num_devices=N_CORES)

    tbl = nc.dram_tensor("tbl", [VPAD, E], f32, kind="ExternalInput")
    wemb = nc.dram_tensor("wemb", [1, E], f32, kind="ExternalInput")
    wvoc = nc.dram_tensor("wvoc", [128, KC], f32, kind="ExternalInput")
    pmat = nc.dram_tensor("pmat", [128, KC, B], fp8, kind="ExternalInput")
    wdup = nc.dram_tensor("wdup", [128, 1], f32, kind="ExternalInput")
    qmat = nc.dram_tensor("qmat", [128, B], fp8, kind="ExternalInput")
    bias = nc.dram_tensor("bias", [1, 1], f32, kind="ExternalInput")
    outp = nc.dram_tensor("outp", [RPC, N_CORES], f32, kind="ExternalOutput")

    with tile.TileContext(nc) as tc:
        with tc.tile_pool(name="dram", bufs=1, space="DRAM") as dram, \
             tc.tile_pool(name="sbuf", bufs=1) as sb1, \
             tc.tile_pool(name="ld", bufs=4) as ld, \
             tc.tile_pool(name="scr", bufs=2) as scr, \
             tc.psum_pool(name="psum", bufs=1) as pp:
            partial_d = dram.tile([B], f32)
            allred_d = dram.tile([B], f32, addr_space="Shared")

            # --- small loads (wemb/wvoc first: they gate the STT chain) ---
            wemb_sb = sb1.tile([128, E], f32)
            nc.sync.dma_start(wemb_sb[:], wemb.ap().partition_broadcast(128))
            nc.vector.tensor_scalar_mul(wemb_sb[:], wemb_sb[:], 1.0 / S)
            wvoc_sb = sb1.tile([128, KC], f32)
            nc.sync.dma_start(wvoc_sb[:], wvoc.ap())
            wdup_sb = sb1.tile([128, 1], f32)
            nc.scalar.dma_start(wdup_sb[:], wdup.ap())
            qm = sb1.tile([128, B], fp8)
            nc.scalar.dma_start(qm[:], qmat.ap())
            b_sb = sb1.tile([128, 1], f32)
            nc.scalar.dma_start(b_sb[:], bias.ap().partition_broadcast(128))
            wneg = sb1.tile([128, 1, 1], bf16)
            nc.vector.tensor_scalar_mul(wneg[:, 0, :], wdup_sb[:], -1.0)

            # warm the sigmoid activation table before the critical tail
            warm = scr.tile([128, 1], f32, tag="warm")
            nc.scalar.activation(
                out=warm[:], in_=b_sb[:],
                func=mybir.ActivationFunctionType.Sigmoid, scale=1.0)

            # --- P matrix: stream in k-chunks alongside the table ---
            pm = sb1.tile([128, KC, B], fp8)
            for ch in range(NCHUNK):
                nc.scalar.dma_start(
                    pm[:, TCHUNK * ch:TCHUNK * (ch + 1), :],
                    pmat.ap()[:, TCHUNK * ch:TCHUNK * (ch + 1), :])

            # --- phase 1 + fused phase 2 matmuls per chunk ---
            u = sb1.tile([128, KC], f32)          # t values
            zb = sb1.tile([128, KC, 1], bf16)     # z = t + wvoc, bf16
            psum_a = pp.tile([1, HALF], f32)
            psum_b = pp.tile([1, HALF], f32)
            tview = tbl.ap().rearrange("(p k) e -> p k e", p=128)
            for ch in range(NCHUNK):
                k0 = TCHUNK * ch
                chunk = ld.tile([128, TCHUNK, E], f32, tag="tblchunk")
                nc.sync.dma_start(chunk[:], tview[:, k0:k0 + TCHUNK, :])
                for c in range(TCHUNK):
                    po = scr.tile([128, E], f32, tag="po")
                    nc.vector.scalar_tensor_tensor(
                        out=po[:], in0=chunk[:, c, :], scalar=1.0,
                        in1=wemb_sb[:],
                        op0=mybir.AluOpType.mult, op1=mybir.AluOpType.mult,
                        accum_out=u[:, k0 + c:k0 + c + 1])
                zch = scr.tile([128, TCHUNK], f32, tag="zch")
                nc.vector.tensor_tensor(
                    out=zch[:], in0=u[:, k0:k0 + TCHUNK],
                    in1=wvoc_sb[:, k0:k0 + TCHUNK], op=mybir.AluOpType.add)
                nc.vector.tensor_copy(out=zb[:, k0:k0 + TCHUNK, 0], in_=zch[:])
                for c in range(TCHUNK):
                    k = k0 + c
                    nc.tensor.matmul(
                        psum_a[:], zb[:, k, :], pm[:, k, :HALF],
                        start=(k == 0), stop=False)
                    nc.tensor.matmul(
                        psum_b[:], zb[:, k, :], pm[:, k, HALF:],
                        start=(k == 0), stop=False)

            # dup correction: one more contraction step on the same PSUM
            nc.tensor.matmul(
                psum_a[:], wneg[:, 0, :], qm[:, :HALF],
                start=False, stop=True)
            nc.tensor.matmul(
                psum_b[:], wneg[:, 0, :], qm[:, HALF:],
                start=False, stop=True)

            partial_sb = sb1.tile([1, B], f32)
            nc.vector.tensor_copy(out=partial_sb[:, :HALF], in_=psum_a[:])
            nc.vector.tensor_copy(out=partial_sb[:, HALF:], in_=psum_b[:])
            nc.scalar.dma_start(partial_d[:], partial_sb[:])

            nc.gpsimd.collective_compute(
                "AllReduce",
                mybir.AluOpType.add,
                replica_groups=[list(range(N_CORES))],
                ins=[partial_d.opt()],
                outs=[allred_d.opt()],
            )

            mine = sb1.tile([RPC, N_CORES], f32)
            nc.scalar.dma_start(
                mine[:], allred_d[:].rearrange("(hi p) -> p hi", p=128))
            res = sb1.tile([RPC, N_CORES], f32)
            nc.scalar.activation(
                out=res[:], in_=mine[:],
                func=mybir.ActivationFunctionType.Sigmoid,
                bias=b_sb[:], scale=1.0)
            nc.scalar.dma_start(outp.ap(), res[:])

    nc.compile()
    return nc


def _first_occurrence_mask(xr: np.ndarray) -> np.ndarray:
    """m[i, j] = 1 iff x[i, j] does not appear at any k < j in row i."""
    eq = xr[:, :, None] == xr[:, None, :]          # [rows, S, S]
    dup = np.tril(eq, -1).any(axis=2)              # seen earlier in the row
    return ~dup


def kernel(x, emb_table, W, b):
    global _BUILT, LAST_RUN
    if _BUILT is None:
        _BUILT = _build()
    nc = _BUILT

    x = np.asarray(x)
    emb_table = np.ascontiguousarray(np.asarray(emb_table, dtype=np.float32))
    W = np.asarray(W, dtype=np.float32)
    b = np.asarray(b, dtype=np.float32)

    wemb = np.ascontiguousarray(W[:, :E])                  # [1, E]
    wv_full = W[0, E:]                                     # [V]
    bias_np = b.reshape(1, 1)

    xl = x.astype(np.int64)                                # [B, S]
    dup = ~_first_occurrence_mask(xl)                      # [B, S] bool

    in_maps = []
    for c in range(N_CORES):
        tbl = np.zeros((VPAD, E), dtype=np.float32)
        tbl[:VPC] = emb_table[c * VPC:(c + 1) * VPC]
        wvs = np.zeros(VPAD, dtype=np.float32)
        wvs[:VPC] = wv_full[c * VPC:(c + 1) * VPC]
        wvoc_sh = wvs.reshape(128, KC)                     # v_loc = p*49 + k

        # count matrix for this vocab slice over ALL batch rows
        in_slice = (xl >= c * VPC) & (xl < (c + 1) * VPC)
        rr, jj = np.nonzero(in_slice)
        vloc = xl[rr, jj] - c * VPC
        pcnt = np.zeros((VPAD, B), dtype=np.int32)
        np.add.at(pcnt, (vloc, rr), 1)
        assert pcnt.max() <= 15, "counts exceed exact fp8e4 range"
        pmat_np = pcnt.astype(ml_dtypes.float8_e4m3).reshape(128, KC, B)

        # dup-correction: -W_voc at dup tokens of this core's OWN rows,
        # one PSUM contraction step (host only INDEXES W, no arithmetic)
        rows = slice(c * RPC, (c + 1) * RPC)
        dr, dj = np.nonzero(dup[rows])                     # local row, seq pos
        dv = xl[rows][dr, dj]                              # global vocab id
        assert len(dr) <= 128, "more than 128 dup tokens on one core"
        wdup_np = np.zeros((128, 1), dtype=np.float32)
        qcnt = np.zeros((128, B), dtype=np.int32)
        for s, (r, v) in enumerate(zip(dr, dv)):
            wdup_np[s, 0] = wv_full[v]
            qcnt[s, c * RPC + r] += 1
        assert qcnt.max() <= 15
        qmat_np = qcnt.astype(ml_dtypes.float8_e4m3)

        in_maps.append({
            "tbl": tbl,
            "wemb": wemb,
            "wvoc": wvoc_sh,
            "pmat": pmat_np,
            "wdup": wdup_np,
            "qmat": qmat_np,
            "bias": bias_np,
        })

    LAST_RUN = run_bass_kernel_spmd(nc, in_maps, core_ids=list(range(N_CORES)))
    out = np.concatenate(
        [LAST_RUN.results[c]["outp"][:, c] for c in range(N_CORES)]
    )
    return out.reshape(B, 1)
